# revision 1
# baseline (speedup 1.0000x reference)
"""AttendAndSpell (LSTM + attention decoder) Trainium2 kernel.

Sharding: data-parallel over batch, 4 batches per core x 8 cores.
All weights + encoder projections SBUF-resident; 256 sequential steps
run inside a hardware For_i loop.

Math (per local batch b, step t):
  gates = [y_t; att; h] @ W_cat.T + (b_ih+b_hh)      (rows reordered i,f,o,g)
  c = sig(f)*c + sig(i)*tanh(g);  h = sig(o)*tanh(c)   (sig via 0.5*tanh(0.5x)+0.5)
  q = W_h.T W_s h            (M2 = W_s.T @ W_h precomputed host-side)
  e[l] = q . enc[b,l] + pe[b,l]   with pe = enc @ (W_h.T b_s)
    (equals s_e . h_e up to a per-(b,t) constant which cancels in softmax)
  u = exp(e)  (no max subtraction; |e| <~ 12 -> safe in fp32)
  ctx = sum_l u[l] enc[b,l];  denom = sum_l u[l]
  att = (W_h @ ctx)/denom + b_h
Outputs: h_seq, att_seq (B, S, H) fp32.
"""
import sys
import numpy as np

sys.path.insert(0, "/opt/trn_rl_repo")

import ml_dtypes

BF16 = ml_dtypes.bfloat16

H = 512
E = 256
B = 32
S = 256
L = 1024
NCORES = 8
BL = B // NCORES  # 4 local batches


def build_program(n_steps=S, split_waits=True):
    from contextlib import ExitStack
    import concourse.bass as bass
    import concourse.mybir as mybir
    from concourse import tile

    dt = mybir.dt
    AF = mybir.ActivationFunctionType
    ALU = mybir.AluOpType

    nc = bass.Bass()

    # ---- DRAM params: all bf16 inputs in one tensor, all f32 in another ----
    NF32 = BL * 8 + 4
    NBF = BL * 4 * L + BL * 8 * H + 8 * 2048 + 4 * H + 4 * H + S * 64 + 2 * NF32
    resbf_d = nc.declare_dram_parameter("resbf", [128, NBF], dt.bfloat16, isOutput=False)
    ha_out = nc.declare_dram_parameter("ha_out", [128, S * 32], dt.float32, isOutput=True)

    with ExitStack() as octx:
        # raw (pre-Tile) resident load: Tile's clocks never see this DMA, so
        # the loop back-edge drain doesn't need a DMA-queue wait.
        resbf_t = octx.enter_context(nc.sbuf_tensor([128, NBF], dt.bfloat16))
        ld_sem = octx.enter_context(nc.semaphore("ld_sem"))
        nc.gpsimd.dma_start(resbf_t[:], resbf_d[:]).then_inc(ld_sem, 16)
        for eng in (nc.tensor, nc.vector, nc.scalar, nc.gpsimd, nc.sync):
            eng.wait_ge(ld_sem, 16)
        _run_tile_body(nc, tile, resbf_t, ha_out, n_steps, NBF, NF32)
    if split_waits:
        _split_big_waits(nc)
    return nc


def _run_tile_body(nc, tile, resbf_t, ha_out, n_steps, NBF, NF32):
    from contextlib import ExitStack
    import concourse.bass as bass
    import concourse.mybir as mybir

    dt = mybir.dt
    AF = mybir.ActivationFunctionType
    ALU = mybir.AluOpType

    with tile.TileContext(nc) as tc, ExitStack() as ctx:
        state = ctx.enter_context(tc.tile_pool(name="state", bufs=1))
        work = ctx.enter_context(tc.tile_pool(name="work", bufs=2))
        psum_p = ctx.enter_context(tc.tile_pool(name="ps", bufs=1, space="PSUM"))

        res = ctx.enter_context(tc.tile_pool(name="res", bufs=1))
        ones_k = res.tile([128, 1], dt.bfloat16)
        ones_m = res.tile([1, 128], dt.float32)
        ha_acc = res.tile([128, S * 32], dt.float32)

        resbf_s = resbf_t
        o = 0
        enc_hT_s = resbf_s[:, o:o + BL * 4 * L]; o += BL * 4 * L
        enc_lT_s = resbf_s[:, o:o + BL * 8 * H]; o += BL * 8 * H
        wcat_s = resbf_s[:, o:o + 8 * 2048]; o += 8 * 2048
        m2_s = resbf_s[:, o:o + 4 * H]; o += 4 * H
        wht_s = resbf_s[:, o:o + 4 * H]; o += 4 * H
        gy_s = resbf_s[:, o:o + S * 64]; o += S * 64
        resf_hi = resbf_s[:, o:o + NF32]; o += NF32
        resf_lo = resbf_s[:, o:o + NF32]; o += NF32
        assert o == NBF
        resf_t = res.tile([128, NF32], dt.float32)
        nc.vector.tensor_tensor(resf_t[:], resf_hi, resf_lo, ALU.add)
        pe_s = resf_t[:, 0:BL * 8]
        bh_s = resf_t[:, BL * 8:BL * 8 + 4]

        nc.vector.memset(ones_k[:], 1.0)
        nc.vector.memset(ones_m[:], 1.0)

        # state
        h_bf = state.tile([128, 16], dt.bfloat16)
        att_bf = state.tile([128, 16], dt.bfloat16)
        c32 = state.tile([128, 16], dt.float32)
        nc.vector.memset(h_bf[:], 0.0)
        nc.vector.memset(att_bf[:], 0.0)
        nc.vector.memset(c32[:], 0.0)

        # pre-allocated tiles (no allocation allowed inside For_i)
        psum_g = psum_p.tile([128, 64], dt.float32, tag="pg")
        psum_q = psum_p.tile([128, 16], dt.float32, tag="pq")
        psum_e = psum_p.tile([128, 32], dt.float32, tag="pe")
        psum_d = psum_p.tile([1, 32], dt.float32, tag="pd")
        psum_ctx = psum_p.tile([128, 16], dt.float32, tag="pctx")
        psum_att = psum_p.tile([128, 16], dt.float32, tag="patt")
        psum_r = psum_p.tile([128, 4], dt.float32, tag="pr")
        tifo = work.tile([128, 48], dt.float32, tag="tifo")
        tg = work.tile([128, 16], dt.float32, tag="tg")
        v1 = work.tile([128, 16], dt.float32, tag="v1")
        v2 = work.tile([128, 16], dt.float32, tag="v2")
        v5 = work.tile([128, 16], dt.float32, tag="v5")
        tc_t = work.tile([128, 16], dt.float32, tag="tc")
        q_bf = work.tile([128, 16], dt.bfloat16, tag="qbf")
        u_sb = work.tile([128, 32], dt.bfloat16, tag="usb")
        sums = work.tile([1, 4], dt.float32, tag="sums")
        r_sb = work.tile([1, 4], dt.float32, tag="rsb")
        ctx_bf = work.tile([128, 16], dt.bfloat16, tag="ctxbf")
        r_bc = work.tile([128, 4], dt.float32, tag="rbc")
        ha_a = work.tile([128, 32], dt.float32, tag="ha_a")
        ha_b = work.tile([128, 32], dt.float32, tag="ha_b")
        ha_bufs = [ha_a, ha_b]

        def step_body(iv, par):
            ha = ha_bufs[par]
            h32 = ha[:, 0:16]
            att32 = ha[:, 16:32]
            o_off = nc.snap(iv * 32)
            gy_off = nc.snap(iv * 64)
            # ---------------- gates (recurrent part only) ----------------
            for mt in range(16):
                for kc in range(8):
                    if kc < 4:
                        rhs = att_bf[:, kc * 4:kc * 4 + 4]
                    else:
                        rhs = h_bf[:, (kc - 4) * 4:(kc - 3) * 4]
                    nc.tensor.matmul(
                        psum_g[:, mt * 4:mt * 4 + 4],
                        wcat_s[:, kc * 2048 + mt * 128: kc * 2048 + mt * 128 + 128],
                        rhs, start=(kc == 0), stop=(kc == 7),
                    )
            nc.vector.tensor_tensor(psum_g[:], psum_g[:], gy_s[:, bass.ds(gy_off, 64)], ALU.add)
            nc.scalar.activation(tifo[:], psum_g[:, 0:48], AF.Tanh, scale=0.5)
            nc.scalar.activation(tg[:], psum_g[:, 48:64], AF.Tanh, scale=1.0)
            ti, tf, to = tifo[:, 0:16], tifo[:, 16:32], tifo[:, 32:48]
            nc.vector.tensor_tensor(v1[:], c32[:], tf, ALU.mult)
            nc.vector.tensor_tensor(v2[:], tg[:], ti, ALU.mult)
            nc.vector.tensor_tensor(v1[:], v1[:], v2[:], ALU.add)
            nc.vector.tensor_tensor(v2[:], c32[:], tg[:], ALU.add)
            nc.vector.tensor_tensor(v5[:], v1[:], v2[:], ALU.add)
            nc.vector.tensor_scalar_mul(c32[:], v5[:], 0.5)
            nc.scalar.activation(tc_t[:], v5[:], AF.Tanh, scale=0.5)
            nc.vector.tensor_tensor(v1[:], to, tc_t[:], ALU.mult)
            nc.vector.tensor_tensor(v2[:], v1[:], tc_t[:], ALU.add)
            nc.vector.tensor_scalar_mul(h32, v2[:], 0.5)
            nc.vector.tensor_copy(h_bf[:], h32)
            
            # ---------------- q = M2.T @ h ----------------
            for mt in range(4):
                for kc in range(4):
                    nc.tensor.matmul(
                        psum_q[:, mt * 4:mt * 4 + 4],
                        m2_s[:, kc * H + mt * 128: kc * H + mt * 128 + 128],
                        h_bf[:, kc * 4:kc * 4 + 4],
                        start=(kc == 0), stop=(kc == 3),
                    )
            nc.vector.tensor_copy(q_bf[:], psum_q[:])

            # ---------------- e scores ----------------
            for b in range(BL):
                for lt in range(8):
                    for kc in range(4):
                        nc.tensor.matmul(
                            psum_e[:, b * 8 + lt: b * 8 + lt + 1],
                            enc_hT_s[:, (b * 4 + kc) * L + lt * 128: (b * 4 + kc) * L + lt * 128 + 128],
                            q_bf[:, kc * 4 + b: kc * 4 + b + 1],
                            start=(kc == 0), stop=(kc == 3),
                        )
            nc.vector.tensor_tensor(psum_e[:], psum_e[:], pe_s, ALU.add)
            nc.scalar.activation(u_sb[:], psum_e[:], AF.Exp)

            # ---------------- denom ----------------
            nc.tensor.matmul(psum_d[:], ones_k[:], u_sb[:], start=True, stop=True)
            nc.vector.tensor_reduce(
                sums[:], psum_d[:].rearrange("p (b l) -> p b l", b=4),
                mybir.AxisListType.X, ALU.add,
            )
            nc.vector.reciprocal(r_sb[:], sums[:])

            # ---------------- ctx ----------------
            for b in range(BL):
                for mt in range(4):
                    for kc in range(8):
                        nc.tensor.matmul(
                            psum_ctx[:, mt * 4 + b: mt * 4 + b + 1],
                            enc_lT_s[:, (b * 8 + kc) * H + mt * 128: (b * 8 + kc) * H + mt * 128 + 128],
                            u_sb[:, b * 8 + kc: b * 8 + kc + 1],
                            start=(kc == 0), stop=(kc == 7),
                        )
            nc.vector.tensor_copy(ctx_bf[:], psum_ctx[:])

            # ---------------- att = (W_h @ ctx)/denom + b_h ----------------
            for mt in range(4):
                for kc in range(4):
                    nc.tensor.matmul(
                        psum_att[:, mt * 4:mt * 4 + 4],
                        wht_s[:, kc * H + mt * 128: kc * H + mt * 128 + 128],
                        ctx_bf[:, kc * 4:kc * 4 + 4],
                        start=(kc == 0), stop=(kc == 3),
                    )
            nc.tensor.matmul(psum_r[:], ones_m[:], r_sb[:], start=True, stop=True)
            nc.vector.tensor_copy(r_bc[:], psum_r[:])
            nc.vector.tensor_tensor(
                att32.rearrange("p (m b) -> p m b", b=4),
                psum_att[:].rearrange("p (m b) -> p m b", b=4),
                r_bc[:].unsqueeze(1).broadcast_to([128, 4, 4]),
                ALU.mult,
            )
            nc.vector.tensor_tensor(
                att32.rearrange("p (m b) -> p m b", b=4),
                att32.rearrange("p (m b) -> p m b", b=4),
                bh_s.unsqueeze(2).broadcast_to([128, 4, 4]),
                ALU.add,
            )
            nc.vector.tensor_copy(att_bf[:], att32)
            nc.vector.tensor_copy(ha_acc[:, bass.ds(o_off, 32)], ha[:])

        def unrollable(iv0, unroll):
            for i in range(unroll):
                step_body(iv0 + i, i % 2)

        tc.For_i_unrolled_general(start=0, end=n_steps, step=1,
                                  unrollable_body=unrollable, max_unroll=4)

        # bulk store of accumulated outputs (outside the loop, static AP)
        nc.gpsimd.dma_start(ha_out[:, 0:n_steps * 32], ha_acc[:, 0:n_steps * 32])


def _split_big_waits(nc, max_waits=1):
    """walrus's setupSyncWait rejects >~2 sem waits on one TPB_CTRL
    instruction; split excess waits onto preceding same-engine NoOps."""
    import concourse.mybir as mybir
    for fn in nc.m.functions:
        for blk in fn.blocks:
            insts = blk.instructions
            new_insts = []
            changed = False
            for inst in insts:
                si = getattr(inst, "sync_info", None)
                waits = list(si.on_wait) if si is not None and si.on_wait else []
                if len(waits) > max_waits:
                    changed = True
                    k = 0
                    while len(waits) - k > max_waits:
                        chunk = waits[k:k + max_waits]
                        k += max_waits
                        nop = mybir.InstNoOp(
                            name=f"{inst.name}-wsplit{k}",
                            engine=inst.engine,
                            ins=[], outs=[],
                            sync_info=mybir.SyncInfo(on_wait=chunk, on_update=[]),
                        )
                        new_insts.append(nop)
                    inst.sync_info = mybir.SyncInfo(
                        on_wait=waits[k:], on_update=list(si.on_update))
                new_insts.append(inst)
            if changed:
                blk.instructions = new_insts


def _arrange_inputs(y, encoder_h, W_ih, b_ih, W_hh, b_hh, W_s, b_s, W_h, b_h):
    """Host-side pre-arrangement. Returns per-core in_maps."""
    f32 = np.float32
    y = np.asarray(y, f32)
    enc = np.asarray(encoder_h, f32)
    W_ih = np.asarray(W_ih, f32); b_ih = np.asarray(b_ih, f32)
    W_hh = np.asarray(W_hh, f32); b_hh = np.asarray(b_hh, f32)
    W_s = np.asarray(W_s, f32); b_s = np.asarray(b_s, f32)
    W_h = np.asarray(W_h, f32); b_h = np.asarray(b_h, f32)

    # gate row reorder [i, f, o, g]
    perm = np.concatenate([np.arange(0, H), np.arange(H, 2 * H),
                           np.arange(3 * H, 4 * H), np.arange(2 * H, 3 * H)])
    W_rec = np.concatenate([W_ih[:, E:], W_hh], axis=1)[perm]    # (2048, 1024) [att; h]
    bsum = (b_ih + b_hh)[perm]                                   # (2048,)
    W_y = W_ih[:, :E][perm]                                      # (2048, 256)
    wrecT = np.ascontiguousarray(W_rec.T)                        # (1024, 2048)
    # wcat arr [128, 8*2048]: col = kc*2048 + m
    wcat_a = wrecT.reshape(8, 128, 2048).transpose(1, 0, 2).reshape(128, 8 * 2048).astype(BF16)
    M2 = (W_s.T @ W_h)                                           # (512, 512)
    m2_a = M2.reshape(4, 128, H).transpose(1, 0, 2).reshape(128, 4 * H).astype(BF16)
    WhT = np.ascontiguousarray(W_h.T)                            # (512, 512)
    wht_a = WhT.reshape(4, 128, H).transpose(1, 0, 2).reshape(128, 4 * H).astype(BF16)
    bh_a = b_h.reshape(4, 128).T.astype(f32).copy()              # [128, 4]
    w_vec = W_h.T @ b_s                                          # (512,)
    pe_full = enc @ w_vec                                        # (B, L)

    in_maps = []
    for ci in range(NCORES):
        sl = slice(ci * BL, (ci + 1) * BL)
        enc_c = enc[sl]                                          # (4, L, H)
        y_c = y[sl]                                              # (4, S, E)
        # enc_hT [128, (b*4+kc)*L + l] = enc[b, l, kc*128+p]
        e_ht = enc_c.transpose(0, 2, 1).reshape(BL, 4, 128, L)   # b, kc, p, l
        e_ht = e_ht.transpose(2, 0, 1, 3).reshape(128, BL * 4 * L).astype(BF16)
        # enc_lT [128, (b*8+kc)*H + hh] = enc[b, kc*128+p, hh]
        e_lt = enc_c.reshape(BL, 8, 128, H).transpose(2, 0, 1, 3).reshape(128, BL * 8 * H).astype(BF16)
        # gy [128, t*64 + mt*4 + b] = (y_c @ W_y.T)[b, t, mt*128+p] + bsum[mt*128+p]
        gy = y_c.reshape(BL * S, E) @ W_y.T + bsum               # (BL*S, 2048)
        gy = gy.reshape(BL, S, 16, 128)
        gy_a = gy.transpose(3, 1, 2, 0).reshape(128, S * 64).astype(BF16)
        # pe [128, b*8+lt] = pe_full[cb, lt*128+p]
        pe_a = pe_full[sl].reshape(BL, 8, 128).transpose(2, 0, 1).reshape(128, BL * 8).astype(f32)
        resf = np.concatenate([pe_a, bh_a], axis=1).astype(np.float32)
        resf_hi = resf.astype(BF16)
        resf_lo = (resf - resf_hi.astype(np.float32)).astype(BF16)
        resbf = np.concatenate([e_ht, e_lt, wcat_a, m2_a, wht_a, gy_a, resf_hi, resf_lo], axis=1)
        in_maps.append({
            "resbf": np.ascontiguousarray(resbf),
        })
    return in_maps


def _unarrange_output(arr):
    """[128, S*32] f32 -> h,att each (BL, S, H): col = t*32 + half*16 + mt*4 + b."""
    a = arr.reshape(128, S, 2, 4, 4)             # p, t, half, mt, b
    a = a.transpose(2, 4, 1, 3, 0)               # half, b, t, mt, p
    return np.ascontiguousarray(a.reshape(2, BL, S, H))


_CACHE = {}


def kernel(y, encoder_h, W_ih, b_ih, W_hh, b_hh, W_s, b_s, W_h, b_h):
    from concourse.bass_utils import run_bass_kernel_spmd

    in_maps = _arrange_inputs(y, encoder_h, W_ih, b_ih, W_hh, b_hh,
                              W_s, b_s, W_h, b_h)
    if "nc" not in _CACHE:
        _CACHE["nc"] = build_program(S)
    nc = _CACHE["nc"]
    import os
    kw = {}
    if os.environ.get("BASS_KERNEL_TRACE"):
        kw = dict(trace=True, tmpdir=os.environ.get("BASS_KERNEL_TRACE_DIR") or None)
    out = run_bass_kernel_spmd(nc, in_maps, list(range(NCORES)), **kw)
    _CACHE["last_result"] = out
    h_parts, a_parts = [], []
    for ci in range(NCORES):
        ha = _unarrange_output(np.asarray(out.results[ci]["ha_out"], np.float32))
        h_parts.append(ha[0])
        a_parts.append(ha[1])
    h_seq = np.concatenate(h_parts, axis=0)
    att_seq = np.concatenate(a_parts, axis=0)
    return h_seq, att_seq

